# revision 32
# baseline (speedup 1.0000x reference)
"""Trainium2 Bass kernel for a 16-head causal MHA layer.

Problem: x:[2,2048,1024] f32, wq/wk/wv/wo:[1024,1024] f32 (Linear-style
[out,in] weights), causal softmax attention with 16 heads of dim 64.

Sharding across the 8 NeuronCores: 2-way data parallel over batch x
4-way tensor parallel over heads.  Core c handles batch c//4 and the 4
heads 4*(c%4) .. 4*(c%4)+3 (feature slice of 256 rows of wq/wk/wv and
256 columns of wo).  Each core produces a partial [2048,1024] output
(its 4 heads' contribution, already projected through its wo slice);
the host sums the 4 partials per batch.

Device dataflow (all matmul inputs fp16, fp32 PSUM accumulation; fp8
was tried and measured 2.1% l2 error -- the softmax does not attenuate
relative error since the attention output shrinks by the same sqrt(eff_k)
factor as the injected score noise -- so everything stays fp16):
  - x arrives fp16, striped by 512-token chunks so projections start as
    soon as the first stripe lands; stripe 0 is split across the two
    hardware DMA queues (sync: dc 0-3, scalar: dc 4-7 after wq) so the
    first projection group starts ~1.5us earlier
  - qT/kT = W @ xT in [feat, token] layout; the reference 1/sqrt(64)
    score scale is folded into the exp activation's free scale slot
  - scoresT[k,q] = kT_h.T-block @ qT_h (64-dim contraction, two heads
    packed onto PE row-halves via tile_position), exp on ACT straight
    out of PSUM, causal mask applied only on diagonal blocks via a
    precomputed 0/1 mask multiply
  - out_unnorm.T | l = (v|1).T-block @ expT accumulated over k blocks
    (the appended ones-column yields the softmax denominator l for free)
  - l stays on-chip in a [2, hc, S] row tile; DVE reciprocal, then a
    K=2 selector matmul broadcasts 1/l across the 128 outT partitions
    (row-half per head), one tensor_mul normalizes both heads at once
  - y = outT.T @ woT accumulated over the 256-dim feature slice

Scheduling (the tensor engine is the bottleneck; exp on ACT is
co-critical during the last query chunk):
  - proj/norm/wo work is queued as "fillers" drained one per
    (group, half) inside the attention loops to cover exp latency
  - kT/v projections for the last chunk are deferred into att(3)'s
    early groups (legal: kb 12-15 is only touched by g=3) so the
    filler queue does not run dry there
  - wo for qt 4..11 is split per-512-column half for finer filling
  - norm(3,0) runs as a filler inside att(3,1); only norm(3,1) +
    wo(12..15) trail, with l-transpose DMAs split across the sync and
    scalar queues and PSUM->SBUF casts split across ACT and DVE
"""

import numpy as np

S = 2048          # sequence length (one batch per core)
D = 1024          # model dim
HL = 4            # heads handled per core
DH = 64           # head dim
F = HL * DH       # 256 local features
DC = D // 128     # 8 d_model chunks of 128
FC = F // 128     # 2 feature chunks of 128
NT = S // 128     # 16 token tiles
NQ = S // 512     # 4 query chunks of 512

_CACHE = {}


def _build_program(dbg=False):
    key = ("nc", dbg)
    if key in _CACHE:
        return _CACHE[key]

    import concourse.bacc as bacc
    import concourse.bass as bass
    import concourse.mybir as mybir
    import concourse.tile as tile

    f16 = mybir.dt.float16
    f32 = mybir.dt.float32
    Exp = mybir.ActivationFunctionType.Exp

    nc = bacc.Bacc("TRN2", target_bir_lowering=False, debug=False)

    # x striped by 512-token chunks: xT_d[t5][p, dc, j] = x[t5*512+j, dc*128+p]
    xT_d = nc.dram_tensor("xT", [NQ, 128, DC, 512], f16, kind="ExternalInput")
    wq_d = nc.dram_tensor("wq", [128, DC, F], f16, kind="ExternalInput")
    wk_d = nc.dram_tensor("wk", [128, DC, F], f16, kind="ExternalInput")
    wv_d = nc.dram_tensor("wv", [128, DC, F], f16, kind="ExternalInput")
    wo_d = nc.dram_tensor("wo", [128, FC, D], f16, kind="ExternalInput")
    y_d = nc.dram_tensor("y", [S, D], f16, kind="ExternalOutput")

    with tile.TileContext(nc) as tc:
        with tc.tile_pool(name="const", bufs=1) as cpool, \
             tc.tile_pool(name="dscr", bufs=1,
                          space=bass.MemorySpace.DRAM) as dpool:
            l_dram = dpool.tile([HL * S], f32)
            xT = cpool.tile([128, NQ, DC, 512], f16)
            wq = cpool.tile([128, DC, F], f16)
            wk = cpool.tile([128, DC, F], f16)
            wv = cpool.tile([128, DC, F], f16)
            wo = cpool.tile([128, FC, D], f16)
            mask = cpool.tile([128, 896], f16)
            ident = cpool.tile([128, 128], f16)
            qT = cpool.tile([128, FC, S], f16)
            kT = cpool.tile([128, FC, S], f16)
            v = cpool.tile([128, NT, HL, DH + 1], f16)
            outT = cpool.tile([128, FC, S], f16)
            l_row = cpool.tile([1, HL * S], f32)
            l_row16 = cpool.tile([1, 1024], f16)
            lT = cpool.tile([128, HL * NT], f32)
            recipT16 = cpool.tile([128, HL * NT], f16)

            # loads: sync + scalar are the only hardware-DGE queues; each
            # sustains only ~half the HBM read bandwidth, so the early
            # tensors are split into pieces ordered to match exactly the
            # prologue's matmul consumption order (wq/x first in dc
            # chunks, then wk, then wv/mask for attention chunk 0)
            nc.sync.dma_start(xT[:, 0, 0:2], xT_d[0, :, 0:2])
            nc.scalar.dma_start(wq[:, 0:4], wq_d[:, 0:4])
            nc.sync.dma_start(xT[:, 0, 2:4], xT_d[0, :, 2:4])
            nc.scalar.dma_start(wq[:, 4:DC], wq_d[:, 4:DC])
            nc.sync.dma_start(xT[:, 0, 4:6], xT_d[0, :, 4:6])
            nc.scalar.dma_start(xT[:, 0, 6:DC], xT_d[0, :, 6:DC])
            nc.sync.dma_start(wk[:, 0:4], wk_d[:, 0:4])
            nc.sync.dma_start(wk[:, 4:DC], wk_d[:, 4:DC])
            nc.scalar.dma_start(wv[:], wv_d[:])
            nc.sync.dma_start(xT[:, 1], xT_d[1])
            nc.sync.dma_start(xT[:, 2], xT_d[2])
            nc.scalar.dma_start(xT[:, 3], xT_d[3])
            nc.scalar.dma_start(wo[:], wo_d[:])

            # constants / ones columns for the softmax-denominator trick
            nc.gpsimd.memset(v[:], 1.0)
            # causal mask + identity built on-device during the input-DMA
            # window (the DVE is idle then); removing them from the DMA
            # stream gets wv/x stripes on-chip earlier.
            # mask[p, j] = 1 where (j - p) >= 384; slices of width 512 at
            # offset 384-128*r give the causal mask for a diagonal block
            # at relative position r (k block kb = 4*qc + r vs the
            # 512-wide q chunk qc)
            nc.gpsimd.memset(mask[:], 1.0)
            nc.gpsimd.affine_select(
                mask[:], mask[:], [[1, 896]], mybir.AluOpType.is_ge, 0.0,
                base=-384, channel_multiplier=-1)
            nc.gpsimd.memset(ident[:], 1.0)
            nc.gpsimd.affine_select(
                ident[:], ident[:], [[1, 128]], mybir.AluOpType.is_equal,
                0.0, base=0, channel_multiplier=-1)

            with tc.tile_pool(name="sc_ps", bufs=2,
                              space=bass.MemorySpace.PSUM) as scp, \
                 tc.tile_pool(name="av_ps", bufs=2,
                              space=bass.MemorySpace.PSUM) as avp, \
                 tc.tile_pool(name="ybc_ps", bufs=2,
                              space=bass.MemorySpace.PSUM) as ybcp, \
                 tc.tile_pool(name="p_sb", bufs=6) as ppool, \
                 tc.tile_pool(name="y_sb", bufs=3) as ysb_pool:

                # HAM warmup: dummy matmuls during the input-load window so
                # the PE clock-gate ramps while DMAs land; also pre-trigger
                # the exp ACT table load off the critical path.  Kept short
                # so the warmups do not delay the first real projection.
                warm = ppool.tile([128, 128], f16, tag="warm", bufs=1)
                warm2 = ppool.tile([128, 128], f16, tag="warm2", bufs=1)
                nc.vector.memset(warm[:], 1.0)
                nc.scalar.activation(warm2[:, 0:1], warm[:, 0:1], Exp)
                wps = ybcp.tile([128, 512], f32, tag="ybc", name="warm_ps")
                for _ in range(24):
                    nc.tensor.matmul(
                        wps[:], warm[:],
                        warm[:, 0:1].to_broadcast((128, 512)),
                        start=True, stop=True)

                def proj_qk_group(w_sb, dstT, fc, t5, big=False):
                    # prologue groups ping-pong between the ybc pool and
                    # the (then unused) score pool so the PSUM->SBUF casts
                    # never gate the next group's matmuls
                    if big:
                        ps = scp.tile([128, 1024], f32, tag="sc",
                                      name=f"pps_{t5}_{fc}_"
                                           f"{0 if dstT is qT else 1}")[:, 0:512]
                    else:
                        ps = ybcp.tile([128, 512], f32, tag="ybc",
                                       name=f"ps_{t5}_{fc}_"
                                            f"{0 if dstT is qT else 1}")
                    for dc in range(DC):
                        nc.tensor.matmul(
                            ps[:],
                            w_sb[:, dc, fc * 128:(fc + 1) * 128],
                            xT[:, t5, dc, :],
                            start=(dc == 0), stop=(dc == DC - 1))
                    nc.vector.tensor_copy(
                        dstT[:, fc, t5 * 512:(t5 + 1) * 512], ps[:])

                def proj_qk0():
                    # prologue ordered to track DMA arrivals: the two wq
                    # groups advance dc-pair by dc-pair as stripe-0 chunks
                    # land, then the wk groups run whole (wk arrives
                    # later); live PSUM tiles split across the ybc + (still
                    # unused) score pools
                    gs = []
                    for i, (w_sb, dstT) in enumerate(((wq, qT), (wk, kT))):
                        for fc in range(FC):
                            nm = f"pqk0_{i}_{fc}"
                            if fc == 0:
                                ps = scp.tile([128, 1024], f32, tag="sc",
                                              name=nm)[:, 0:512]
                            else:
                                ps = ybcp.tile([128, 512], f32, tag="ybc",
                                               name=nm)
                            gs.append((w_sb, dstT, fc, ps))

                    def mm(g, dc):
                        w_sb, dstT, fc, ps = g
                        nc.tensor.matmul(
                            ps[:],
                            w_sb[:, dc, fc * 128:(fc + 1) * 128],
                            xT[:, 0, dc, :],
                            start=(dc == 0), stop=(dc == DC - 1))

                    for dc2 in range(0, DC, 2):
                        for g in gs[0:2]:
                            mm(g, dc2)
                            mm(g, dc2 + 1)
                    for g in gs[0:2]:
                        w_sb, dstT, fc, ps = g
                        nc.vector.tensor_copy(dstT[:, fc, 0:512], ps[:])
                    for g in gs[2:4]:
                        for dc in range(DC):
                            mm(g, dc)
                    for g in gs[2:4]:
                        w_sb, dstT, fc, ps = g
                        nc.vector.tensor_copy(dstT[:, fc, 0:512], ps[:])

                def proj_v_group(tt, big=False):
                    t5, r = divmod(tt, 4)
                    if big:
                        psv = scp.tile([128, 1024], f32, tag="sc",
                                       name=f"ppsv_{tt}")[:, 0:F]
                    else:
                        psv = ybcp.tile([128, F], f32, tag="ybc",
                                        name=f"psv_{tt}")
                    for dc in range(DC):
                        nc.tensor.matmul(
                            psv[:],
                            xT[:, t5, dc, r * 128:(r + 1) * 128],
                            wv[:, dc, :],
                            start=(dc == 0), stop=(dc == DC - 1))
                    nc.vector.tensor_copy(
                        v[:, tt, :, 0:DH],
                        psv.rearrange("p (h d) -> p h d", h=HL))

                import collections
                fillers = collections.deque()

                def run_filler(n):
                    for _ in range(n):
                        if fillers:
                            fillers.popleft()()

                def att_hc(qc, hc):
                    last = (qc == NQ - 1)
                    avs = []
                    for hp2 in range(2):
                        av = avp.tile([DH + 1, 512], f32, tag="av",
                                      name=f"av_{hc}_{qc}_{hp2}")
                        avs.append(av)
                    for g in range(qc + 1):
                        diag = (g == qc)
                        for half in range(2):
                            # (offset, width) of each k-block's valid
                            # q-span inside the p tile; diagonal blocks
                            # are clipped to q >= k_block_start
                            if diag:
                                rs = [2 * half, 2 * half + 1]
                                spans = [(128 * r, 512 - 128 * r)
                                         for r in rs]
                            else:
                                spans = [(0, 512), (0, 512)]
                            offs = [0, spans[0][1]]
                            scs = []
                            for hp2 in range(2):
                                sc = scp.tile([128, 1024], f32, tag="sc",
                                              name=f"sc_{hc}_{qc}_{g}_{half}_{hp2}")
                                scs.append(sc)
                            for r2 in range(2):
                                kb = 4 * g + 2 * half + r2
                                qo, w = spans[r2]
                                for hp2 in range(2):
                                    hp = hp2 * 64
                                    nc.tensor.matmul(
                                        scs[hp2][:, offs[r2]:offs[r2] + w],
                                        kT[hp:hp + 64, hc,
                                           kb * 128:(kb + 1) * 128],
                                        qT[hp:hp + 64, hc,
                                           qc * 512 + qo:(qc + 1) * 512],
                                        start=True, stop=True,
                                        tile_position=(hp, 0))
                            width = offs[1] + spans[1][1]
                            for hp2 in range(2):
                                h = hc * 2 + hp2
                                p_sb = ppool.tile([128, 1024], f16,
                                                  tag=f"p{hp2}",
                                                  name=f"p_{hc}_{qc}_{g}_{half}_{hp2}")
                                # the reference 1/sqrt(64) score scale
                                nc.scalar.activation(
                                    p_sb[:, 0:width],
                                    scs[hp2][:, 0:width], Exp,
                                    scale=0.125)
                                if diag:
                                    # only the first 128 columns of a
                                    # clipped block straddle the diagonal
                                    for r2 in range(2):
                                        nc.vector.tensor_mul(
                                            p_sb[:, offs[r2]:offs[r2] + 128],
                                            p_sb[:, offs[r2]:offs[r2] + 128],
                                            mask[:, 384:512])
                                for r2 in range(2):
                                    kb = 4 * g + 2 * half + r2
                                    qo, w = spans[r2]
                                    nc.tensor.matmul(
                                        avs[hp2][:, qo:512],
                                        v[:, kb, h, :],
                                        p_sb[:, offs[r2]:offs[r2] + w],
                                        start=(kb == 0),
                                        stop=(kb == 4 * qc + 3))
                            # hold the last two fillers back on the final
                            # diagonal group: they instead bridge the
                            # tensor-idle window between the last AV and
                            # the tail norm/wo chain (an idle dip there
                            # also triggers a ~10us PE half-clock clamp)
                            if not (last and hc == 1 and diag):
                                run_filler(1)
                    if last and hc == 1:
                        # tail epilogue: the l-row copies go FIRST (the
                        # K=1 transpose matmuls below wait on them), split
                        # across DVE and ACT so they run in parallel; the
                        # outT copies follow (their consumers come later)
                        nc.vector.tensor_copy(
                            l_row16[0:1, 0:512], avs[0][DH:DH + 1, :])
                        nc.scalar.copy(
                            l_row16[0:1, 512:1024], avs[1][DH:DH + 1, :])
                        # held-back fillers go on the tensor queue first
                        # so the PE isn't idle while the copies land
                        run_filler(len(fillers))
                        # lT_ps[p, hp2*4+t] = l_row16[0, hp2*512+128t+p]:
                        # K=1 matmul with the single l row as stationary
                        # and a 1.0 scalar as the moving operand; PSUM
                        # comes from the score pool (idle from here on)
                        # so the ybc ring stays free for the tail wo
                        ltp = scp.tile([128, 1024], f32, tag="sc",
                                       name="ltp")[:, 0:8]
                        for hp2 in range(2):
                            for t4 in range(4):
                                nc.tensor.matmul(
                                    ltp[:, hp2 * 4 + t4:hp2 * 4 + t4 + 1],
                                    l_row16[0:1, hp2 * 512 + t4 * 128:
                                            hp2 * 512 + (t4 + 1) * 128],
                                    warm[0:1, 0:1],
                                    start=True, stop=True)
                        att_hc.ltp = ltp
                        # reciprocals immediately (the bc broadcast
                        # matmuls in the tail norm wait on them)
                        with nc.allow_low_precision(
                                reason="fp16 1/l; l>=1 so ~5e-4 relative"):
                            for hp2 in range(2):
                                h = hc * 2 + hp2
                                cols = slice(h * NT + 4 * qc,
                                             h * NT + 4 * qc + 4)
                                nc.vector.reciprocal(
                                    recipT16[:, cols],
                                    ltp[:, hp2 * 4:hp2 * 4 + 4])
                        # outT copies in 256-column halves, split across
                        # ACT/DVE, so the first tail normalization chunk
                        # isn't gated on a whole 512-wide copy
                        for ci in range(2):
                            cs = slice(qc * 512 + ci * 256,
                                       qc * 512 + (ci + 1) * 256)
                            vs = slice(ci * 256, (ci + 1) * 256)
                            nc.scalar.copy(
                                outT[0:64, hc, cs], avs[0][0:DH, vs])
                            nc.vector.tensor_copy(
                                outT[64:128, hc, cs], avs[1][0:DH, vs])
                    else:
                        for hp2 in range(2):
                            h = hc * 2 + hp2
                            nc.vector.tensor_copy(
                                outT[hp2 * 64:hp2 * 64 + 64, hc,
                                     qc * 512:(qc + 1) * 512],
                                avs[hp2][0:DH, :])
                            # denominators: need the 512 l values spread
                            # across 128 partitions (a [1,512] single-lane
                            # DVE reciprocal measures 3.3us; the [128,4]
                            # one is ~0.15us); the roundtrip through DRAM
                            # is fully overlapped in steady state
                            seg = slice(h * S + qc * 512,
                                        h * S + (qc + 1) * 512)
                            nc.vector.tensor_copy(
                                l_row[0:1, seg], avs[hp2][DH:DH + 1, :])
                            nc.sync.dma_start(l_dram[seg], l_row[0:1, seg])
                            nc.sync.dma_start(
                                lT[:, h * NT + 4 * qc:h * NT + 4 * qc + 4],
                                l_dram[seg].rearrange("(t p) -> p t", p=128))

                def norm_pair(qc, hc):
                    # 1/l on the [q-partition] transposed copy (128 DVE
                    # lanes), broadcast over the dh rows with K=128 ident
                    # matmuls -- the two heads packed onto PE column halves
                    # via tile_position -- then one tensor_mul normalizes
                    # the whole [128,512] chunk
                    sl = slice(qc * 512, (qc + 1) * 512)
                    from_ltp = (qc == NQ - 1 and hc == 1)
                    if not from_ltp:
                        with nc.allow_low_precision(
                                reason="fp16 1/l; l>=1 so ~5e-4 relative"):
                            for hp2 in range(2):
                                h = hc * 2 + hp2
                                cols = slice(h * NT + 4 * qc,
                                             h * NT + 4 * qc + 4)
                                nc.vector.reciprocal(recipT16[:, cols],
                                                     lT[:, cols])
                    if from_ltp:
                        # the ybc ring is reserved for the tail wo PSUM;
                        # the score pool is idle from here on
                        bc = scp.tile([128, 1024], f32, tag="sc",
                                      name=f"bc_{hc}_{qc}")[:, 0:512]
                    else:
                        bc = ybcp.tile([128, 512], f32, tag="ybc",
                                       name=f"bc_{hc}_{qc}")
                    for hp2 in range(2):
                        for t4 in range(4):
                            col = (hc * 2 + hp2) * NT + 4 * qc + t4
                            nc.tensor.matmul(
                                bc[hp2 * 64:(hp2 + 1) * 64,
                                   t4 * 128:(t4 + 1) * 128],
                                recipT16[:, col:col + 1]
                                .to_broadcast((128, DH)),
                                ident[:],
                                start=True, stop=True,
                                tile_position=(0, hp2 * 64))
                    if from_ltp:
                        # final chunk: normalize in 128-column pieces so
                        # the first tail wo matmul isn't gated on the
                        # whole 512-wide multiply
                        for t4 in range(4):
                            s4 = slice(qc * 512 + t4 * 128,
                                       qc * 512 + (t4 + 1) * 128)
                            nc.vector.tensor_mul(
                                outT[:, hc, s4], outT[:, hc, s4],
                                bc[:, t4 * 128:(t4 + 1) * 128])
                    else:
                        nc.vector.tensor_mul(
                            outT[:, hc, sl], outT[:, hc, sl], bc[:])

                ysb_map = {}

                def wo_oc(qt, oc, tail=False):
                    if qt not in ysb_map:
                        ysb_map[qt] = ysb_pool.tile(
                            [128, 1024], f16, tag="ysb", name=f"ysb_{qt}")
                    ysb = ysb_map[qt]
                    yps = ybcp.tile([128, 512], f32, tag="ybc",
                                    name=f"yps_{qt}_{oc}")
                    for fc in range(FC):
                        nc.tensor.matmul(
                            yps[:],
                            outT[:, fc, qt * 128:(qt + 1) * 128],
                            wo[:, fc, oc * 512:(oc + 1) * 512],
                            start=(fc == 0), stop=(fc == FC - 1))
                    if tail and oc == 0:
                        nc.scalar.copy(
                            ysb[:, oc * 512:(oc + 1) * 512], yps[:])
                    else:
                        nc.vector.tensor_copy(
                            ysb[:, oc * 512:(oc + 1) * 512], yps[:])
                    if tail and qt == 4 * NQ - 1:
                        # last token block: ship each half as soon as its
                        # cast lands so the final DMA isn't serialized
                        # behind both halves
                        nc.sync.dma_start(
                            y_d[qt * 128:(qt + 1) * 128,
                                oc * 512:(oc + 1) * 512],
                            ysb[:, oc * 512:(oc + 1) * 512])
                    elif oc == 1:
                        nc.sync.dma_start(
                            y_d[qt * 128:(qt + 1) * 128, :], ysb[:])

                def wo_qt(qt, tail=False):
                    for oc in range(2):
                        wo_oc(qt, oc, tail=tail)

                proj_qk0()
                for tt in range(4):
                    proj_v_group(tt, big=(tt % 2 == 0))
                for qc in range(NQ):
                    if qc + 1 < NQ:
                        nxt = qc + 1
                        if nxt < NQ - 1:
                            # project everything for the next chunk now
                            for w_sb, dstT in ((wq, qT), (wk, kT)):
                                for fc in range(FC):
                                    fillers.append(
                                        lambda w=w_sb, d=dstT, f=fc, t=nxt:
                                        proj_qk_group(w, d, f, t))
                            for tt in range(4 * nxt, 4 * nxt + 4):
                                fillers.append(lambda t=tt: proj_v_group(t))
                        else:
                            # last chunk: only qT is needed at att(3) g=0;
                            # kT + v for kb 12-15 are deferred into att(3)
                            # itself (first touched at g=3) to keep the
                            # filler queue alive there
                            for fc in range(FC):
                                fillers.append(
                                    lambda f=fc, t=nxt:
                                    proj_qk_group(wq, qT, f, t))
                    if qc >= 1:
                        for hcx in range(FC):
                            fillers.append(
                                lambda q=qc - 1, c=hcx: norm_pair(q, c))
                        if qc == 1:
                            for qt in range(0, 4):
                                fillers.append(lambda a=qt: wo_qt(a))
                        else:
                            for qt in range(4 * (qc - 1), 4 * qc):
                                for oc in range(2):
                                    fillers.append(
                                        lambda a=qt, o=oc: wo_oc(a, o))
                    if qc == NQ - 1:
                        # deferred last-chunk projections, due before g=3
                        deferred = []
                        for fc in range(FC):
                            deferred.append(
                                lambda f=fc, t=qc:
                                proj_qk_group(wk, kT, f, t))
                        for tt in range(4 * qc, 4 * qc + 4):
                            deferred.append(lambda t=tt: proj_v_group(t))
                        fillers.extendleft(reversed(deferred))
                    att_hc(qc, 0)
                    if qc == NQ - 1:
                        # norm(3,0) inside att(3,1), late enough that its
                        # l-transpose DMAs (issued at the end of att(3,0))
                        # have landed by the time the reciprocal runs
                        fillers.insert(min(4, len(fillers)),
                                       lambda: norm_pair(NQ - 1, 0))
                    att_hc(qc, 1)
                    run_filler(len(fillers))
                # tail wo: interleave token-block pairs so a ready fc0
                # matmul always sits between the normalization-gated fc1
                # matmuls; casts alternate ACT/DVE; the last block ships
                # per-half.  The first pair's fc0 matmuls are hoisted
                # ahead of the tail norm so the PE stays busy while the
                # reciprocals land.
                for qt in range(4 * (NQ - 1), 4 * NQ):
                    ysb_map[qt] = ysb_pool.tile([128, 1024], f16,
                                                tag="ysb", name=f"ysb_{qt}")
                yp = {}

                def wo_fc0(qt, oc, pool="ybc"):
                    if pool == "sc":
                        yp[(qt, oc)] = scp.tile(
                            [128, 1024], f32, tag="sc",
                            name=f"ypt_{qt}_{oc}")[:, 0:512]
                    else:
                        yp[(qt, oc)] = ybcp.tile(
                            [128, 512], f32, tag="ybc",
                            name=f"ypt_{qt}_{oc}")
                    nc.tensor.matmul(
                        yp[(qt, oc)][:],
                        outT[:, 0, qt * 128:(qt + 1) * 128],
                        wo[:, 0, oc * 512:(oc + 1) * 512],
                        start=True, stop=False)

                b0 = 4 * (NQ - 1)
                wo_fc0(b0, 0)
                wo_fc0(b0 + 1, 0)
                norm_pair(NQ - 1, 1)
                # third and fourth fc0 matmuls ride the score pool (its
                # ltp/bc slots are read-complete by then), so the PE has
                # ready work while the per-chunk normalizations drain
                wo_fc0(b0 + 2, 0, pool="sc")
                wo_fc0(b0 + 3, 0, pool="sc")
                for base in (b0, b0 + 2):
                    for oc in range(2):
                        for qt in (base, base + 1):
                            if (qt, oc) not in yp:
                                wo_fc0(qt, oc)
                        for qt in (base, base + 1):
                            nc.tensor.matmul(
                                yp[(qt, oc)][:],
                                outT[:, 1, qt * 128:(qt + 1) * 128],
                                wo[:, 1, oc * 512:(oc + 1) * 512],
                                start=False, stop=True)
                        for qt in (base, base + 1):
                            dst = ysb_map[qt][:, oc * 512:(oc + 1) * 512]
                            if qt == 4 * NQ - 1 and oc == 1:
                                # final piece: 256-wide halves on both
                                # engines, shipped separately, so the
                                # last DMA starts as early as possible
                                nc.scalar.copy(
                                    dst[:, 0:256], yp[(qt, oc)][:, 0:256])
                                nc.vector.tensor_copy(
                                    dst[:, 256:512],
                                    yp[(qt, oc)][:, 256:512])
                                nc.sync.dma_start(
                                    y_d[qt * 128:(qt + 1) * 128,
                                        512:768], dst[:, 0:256])
                                nc.sync.dma_start(
                                    y_d[qt * 128:(qt + 1) * 128,
                                        768:1024], dst[:, 256:512])
                                continue
                            if (qt + oc) % 2 == 0:
                                nc.scalar.copy(dst, yp[(qt, oc)][:])
                            else:
                                nc.vector.tensor_copy(dst, yp[(qt, oc)][:])
                            if qt == 4 * NQ - 1:
                                nc.sync.dma_start(
                                    y_d[qt * 128:(qt + 1) * 128,
                                        oc * 512:(oc + 1) * 512], dst)
                            elif oc == 1:
                                nc.sync.dma_start(
                                    y_d[qt * 128:(qt + 1) * 128, :],
                                    ysb_map[qt][:])

    nc.compile()

    from concourse.bass_interp import get_hw_module
    nc.m = get_hw_module(nc.m)

    _CACHE[key] = nc
    return nc


def kernel(x, wq, wk, wv, wo):
    x = np.asarray(x, dtype=np.float32)
    wq = np.asarray(wq, dtype=np.float32)
    wk = np.asarray(wk, dtype=np.float32)
    wv = np.asarray(wv, dtype=np.float32)
    wo = np.asarray(wo, dtype=np.float32)

    from concourse import bass_utils

    nc = _build_program()

    def sbuf_w(w):
        # [out=256, in=1024] -> [128, DC, 256] SBUF layout, contiguous DMA
        return np.ascontiguousarray(
            w.T.reshape(DC, 128, F).transpose(1, 0, 2)).astype(np.float16)

    in_maps = []
    for c in range(8):
        b = c // 4
        hg = c % 4
        fs = slice(hg * F, (hg + 1) * F)
        # [NQ, 128, DC, 512]: stripe-major for early projection start
        xT = np.ascontiguousarray(
            x[b].T.reshape(DC, 128, NQ, 512).transpose(2, 1, 0, 3)
        ).astype(np.float16)
        woT = np.ascontiguousarray(
            wo[:, fs].T.reshape(FC, 128, D).transpose(1, 0, 2)
        ).astype(np.float16)
        in_maps.append({
            "xT": xT,
            "wq": sbuf_w(wq[fs, :]),
            "wk": sbuf_w(wk[fs, :]),
            "wv": sbuf_w(wv[fs, :]),
            "wo": woT,
        })

    res = bass_utils.run_bass_kernel_spmd(nc, in_maps, core_ids=list(range(8)))
    ys = [res.results[c]["y"].astype(np.float32) for c in range(8)]
    out = np.stack([ys[0] + ys[1] + ys[2] + ys[3],
                    ys[4] + ys[5] + ys[6] + ys[7]])
    return out


# revision 37
# speedup vs baseline: 1.0108x; 1.0108x over previous
"""Trainium2 Bass kernel for a 16-head causal MHA layer.

Problem: x:[2,2048,1024] f32, wq/wk/wv/wo:[1024,1024] f32 (Linear-style
[out,in] weights), causal softmax attention with 16 heads of dim 64.

Sharding across the 8 NeuronCores: 2-way data parallel over batch x
4-way tensor parallel over heads.  Core c handles batch c//4 and the 4
heads 4*(c%4) .. 4*(c%4)+3 (feature slice of 256 rows of wq/wk/wv and
256 columns of wo).  Each core produces a partial [2048,1024] output
(its 4 heads' contribution, already projected through its wo slice);
the host sums the 4 partials per batch.

Device dataflow (all matmul inputs fp16, fp32 PSUM accumulation; fp8
was tried and measured 2.1% l2 error -- the softmax does not attenuate
relative error since the attention output shrinks by the same sqrt(eff_k)
factor as the injected score noise -- so everything stays fp16):
  - x arrives fp16, striped by 512-token chunks so projections start as
    soon as the first stripe lands; stripe 0 is split across the two
    hardware DMA queues (sync: dc 0-3, scalar: dc 4-7 after wq) so the
    first projection group starts ~1.5us earlier
  - qT/kT = W @ xT in [feat, token] layout; the reference 1/sqrt(64)
    score scale is folded into the exp activation's free scale slot
  - scoresT[k,q] = kT_h.T-block @ qT_h (64-dim contraction, two heads
    packed onto PE row-halves via tile_position), exp on ACT straight
    out of PSUM, causal mask applied only on diagonal blocks via a
    precomputed 0/1 mask multiply
  - out_unnorm.T | l = (v|1).T-block @ expT accumulated over k blocks
    (the appended ones-column yields the softmax denominator l for free)
  - l stays on-chip in a [2, hc, S] row tile; DVE reciprocal, then a
    K=2 selector matmul broadcasts 1/l across the 128 outT partitions
    (row-half per head), one tensor_mul normalizes both heads at once
  - y = outT.T @ woT accumulated over the 256-dim feature slice

Scheduling (the tensor engine is the bottleneck; exp on ACT is
co-critical during the last query chunk):
  - proj/norm/wo work is queued as "fillers" drained one per
    (group, half) inside the attention loops to cover exp latency
  - kT/v projections for the last chunk are deferred into att(3)'s
    early groups (legal: kb 12-15 is only touched by g=3) so the
    filler queue does not run dry there
  - wo for qt 4..11 is split per-512-column half for finer filling
  - norm(3,0) runs as a filler inside att(3,1); only norm(3,1) +
    wo(12..15) trail, with l-transpose DMAs split across the sync and
    scalar queues and PSUM->SBUF casts split across ACT and DVE
"""

import numpy as np

S = 2048          # sequence length (one batch per core)
D = 1024          # model dim
HL = 4            # heads handled per core
DH = 64           # head dim
F = HL * DH       # 256 local features
DC = D // 128     # 8 d_model chunks of 128
FC = F // 128     # 2 feature chunks of 128
NT = S // 128     # 16 token tiles
NQ = S // 512     # 4 query chunks of 512

_CACHE = {}


def _build_program(dbg=False):
    key = ("nc", dbg)
    if key in _CACHE:
        return _CACHE[key]

    import concourse.bacc as bacc
    import concourse.bass as bass
    import concourse.mybir as mybir
    import concourse.tile as tile

    f16 = mybir.dt.float16
    f32 = mybir.dt.float32
    Exp = mybir.ActivationFunctionType.Exp

    nc = bacc.Bacc("TRN2", target_bir_lowering=False, debug=False)

    # x striped by 512-token chunks: xT_d[t5][p, dc, j] = x[t5*512+j, dc*128+p]
    xT_d = nc.dram_tensor("xT", [NQ, 128, DC, 512], f16, kind="ExternalInput")
    wq_d = nc.dram_tensor("wq", [128, DC, F], f16, kind="ExternalInput")
    wk_d = nc.dram_tensor("wk", [128, DC, F], f16, kind="ExternalInput")
    wv_d = nc.dram_tensor("wv", [128, DC, F], f16, kind="ExternalInput")
    wo_d = nc.dram_tensor("wo", [128, FC, D], f16, kind="ExternalInput")
    y_d = nc.dram_tensor("y", [S, D], f16, kind="ExternalOutput")

    with tile.TileContext(nc) as tc:
        with tc.tile_pool(name="const", bufs=1) as cpool, \
             tc.tile_pool(name="dscr", bufs=1,
                          space=bass.MemorySpace.DRAM) as dpool:
            l_dram = dpool.tile([HL * S], f32)
            xT = cpool.tile([128, NQ, DC, 512], f16)
            wq = cpool.tile([128, DC, F], f16)
            wk = cpool.tile([128, DC, F], f16)
            wv = cpool.tile([128, DC, F], f16)
            wo = cpool.tile([128, FC, D], f16)
            mask = cpool.tile([128, 896], f16)
            ident = cpool.tile([128, 128], f16)
            qT = cpool.tile([128, FC, S], f16)
            kT = cpool.tile([128, FC, S], f16)
            v = cpool.tile([128, NT, HL, DH + 1], f16)
            outT = cpool.tile([128, FC, S], f16)
            l_row = cpool.tile([1, HL * S], f32)
            l_row16 = cpool.tile([1, 1024], f16)
            lT = cpool.tile([128, HL * NT], f32)
            recipT16 = cpool.tile([128, HL * NT], f16)

            # loads: sync + scalar are the only hardware-DGE queues; each
            # sustains only ~half the HBM read bandwidth, so the early
            # tensors are split into pieces ordered to match exactly the
            # prologue's matmul consumption order (wq/x first in dc
            # chunks, then wk, then wv/mask for attention chunk 0)
            nc.sync.dma_start(xT[:, 0, 0:2], xT_d[0, :, 0:2])
            nc.scalar.dma_start(wq[:, 0:4], wq_d[:, 0:4])
            nc.sync.dma_start(xT[:, 0, 2:4], xT_d[0, :, 2:4])
            nc.scalar.dma_start(wq[:, 4:DC], wq_d[:, 4:DC])
            nc.sync.dma_start(xT[:, 0, 4:6], xT_d[0, :, 4:6])
            nc.scalar.dma_start(xT[:, 0, 6:DC], xT_d[0, :, 6:DC])
            nc.sync.dma_start(wk[:, 0:4], wk_d[:, 0:4])
            nc.sync.dma_start(wk[:, 4:DC], wk_d[:, 4:DC])
            nc.scalar.dma_start(wv[:], wv_d[:])
            nc.sync.dma_start(xT[:, 1], xT_d[1])
            nc.sync.dma_start(xT[:, 2], xT_d[2])
            nc.scalar.dma_start(xT[:, 3], xT_d[3])
            nc.scalar.dma_start(wo[:], wo_d[:])

            # constants / ones columns for the softmax-denominator trick
            nc.gpsimd.memset(v[:], 1.0)
            # causal mask + identity built on-device during the input-DMA
            # window (the DVE is idle then); removing them from the DMA
            # stream gets wv/x stripes on-chip earlier.
            # mask[p, j] = 1 where (j - p) >= 384; slices of width 512 at
            # offset 384-128*r give the causal mask for a diagonal block
            # at relative position r (k block kb = 4*qc + r vs the
            # 512-wide q chunk qc)
            nc.gpsimd.memset(mask[:], 1.0)
            nc.gpsimd.affine_select(
                mask[:], mask[:], [[1, 896]], mybir.AluOpType.is_ge, 0.0,
                base=-384, channel_multiplier=-1)
            nc.gpsimd.memset(ident[:], 1.0)
            nc.gpsimd.affine_select(
                ident[:], ident[:], [[1, 128]], mybir.AluOpType.is_equal,
                0.0, base=0, channel_multiplier=-1)

            with tc.tile_pool(name="sc_ps", bufs=2,
                              space=bass.MemorySpace.PSUM) as scp, \
                 tc.tile_pool(name="av_ps", bufs=2,
                              space=bass.MemorySpace.PSUM) as avp, \
                 tc.tile_pool(name="ybc_ps", bufs=2,
                              space=bass.MemorySpace.PSUM) as ybcp, \
                 tc.tile_pool(name="p_sb", bufs=6) as ppool, \
                 tc.tile_pool(name="y_sb", bufs=3) as ysb_pool:

                # HAM warmup: dummy matmuls during the input-load window so
                # the PE clock-gate ramps while DMAs land; also pre-trigger
                # the exp ACT table load off the critical path.  Kept short
                # so the warmups do not delay the first real projection.
                warm = ppool.tile([128, 128], f16, tag="warm", bufs=1)
                warm2 = ppool.tile([128, 128], f16, tag="warm2", bufs=1)
                nc.vector.memset(warm[:], 1.0)
                nc.scalar.activation(warm2[:, 0:1], warm[:, 0:1], Exp)
                wps = ybcp.tile([128, 512], f32, tag="ybc", name="warm_ps")
                for _ in range(24):
                    nc.tensor.matmul(
                        wps[:], warm[:],
                        warm[:, 0:1].to_broadcast((128, 512)),
                        start=True, stop=True)

                def proj_qk_group(w_sb, dstT, fc, t5, big=False):
                    # prologue groups ping-pong between the ybc pool and
                    # the (then unused) score pool so the PSUM->SBUF casts
                    # never gate the next group's matmuls
                    if big:
                        ps = scp.tile([128, 1024], f32, tag="sc",
                                      name=f"pps_{t5}_{fc}_"
                                           f"{0 if dstT is qT else 1}")[:, 0:512]
                    else:
                        ps = ybcp.tile([128, 512], f32, tag="ybc",
                                       name=f"ps_{t5}_{fc}_"
                                            f"{0 if dstT is qT else 1}")
                    for dc in range(DC):
                        nc.tensor.matmul(
                            ps[:],
                            w_sb[:, dc, fc * 128:(fc + 1) * 128],
                            xT[:, t5, dc, :],
                            start=(dc == 0), stop=(dc == DC - 1))
                    nc.vector.tensor_copy(
                        dstT[:, fc, t5 * 512:(t5 + 1) * 512], ps[:])

                def proj_qk0():
                    # prologue ordered to track DMA arrivals: the two wq
                    # groups advance dc-pair by dc-pair as stripe-0 chunks
                    # land, then the wk groups run whole (wk arrives
                    # later); live PSUM tiles split across the ybc + (still
                    # unused) score pools
                    gs = []
                    for i, (w_sb, dstT) in enumerate(((wq, qT), (wk, kT))):
                        for fc in range(FC):
                            nm = f"pqk0_{i}_{fc}"
                            if fc == 0:
                                ps = scp.tile([128, 1024], f32, tag="sc",
                                              name=nm)[:, 0:512]
                            else:
                                ps = ybcp.tile([128, 512], f32, tag="ybc",
                                               name=nm)
                            gs.append((w_sb, dstT, fc, ps))

                    def mm(g, dc):
                        w_sb, dstT, fc, ps = g
                        nc.tensor.matmul(
                            ps[:],
                            w_sb[:, dc, fc * 128:(fc + 1) * 128],
                            xT[:, 0, dc, :],
                            start=(dc == 0), stop=(dc == DC - 1))

                    # casts split across DVE and ACT (ACT is idle until
                    # the first attention exp) so the eight prologue
                    # casts don't serialize on the vector engine
                    for dc2 in range(0, DC, 2):
                        for g in gs[0:2]:
                            mm(g, dc2)
                            mm(g, dc2 + 1)
                    for i, g in enumerate(gs[0:2]):
                        w_sb, dstT, fc, ps = g
                        if i == 0:
                            nc.vector.tensor_copy(dstT[:, fc, 0:512], ps[:])
                        else:
                            nc.scalar.copy(dstT[:, fc, 0:512], ps[:])
                    for g in gs[2:4]:
                        for dc in range(DC):
                            mm(g, dc)
                    for i, g in enumerate(gs[2:4]):
                        w_sb, dstT, fc, ps = g
                        if i == 0:
                            nc.vector.tensor_copy(dstT[:, fc, 0:512], ps[:])
                        else:
                            nc.scalar.copy(dstT[:, fc, 0:512], ps[:])

                def proj_v_group(tt, big=False, cast_eng="vector"):
                    t5, r = divmod(tt, 4)
                    if big:
                        psv = scp.tile([128, 1024], f32, tag="sc",
                                       name=f"ppsv_{tt}")[:, 0:F]
                    else:
                        psv = ybcp.tile([128, F], f32, tag="ybc",
                                        name=f"psv_{tt}")
                    for dc in range(DC):
                        nc.tensor.matmul(
                            psv[:],
                            xT[:, t5, dc, r * 128:(r + 1) * 128],
                            wv[:, dc, :],
                            start=(dc == 0), stop=(dc == DC - 1))
                    nc.vector.tensor_copy(
                        v[:, tt, :, 0:DH],
                        psv.rearrange("p (h d) -> p h d", h=HL))

                import collections
                fillers = collections.deque()

                def run_filler(n):
                    for _ in range(n):
                        if fillers:
                            fillers.popleft()()

                def att_hc(qc, hc):
                    last = (qc == NQ - 1)
                    avs = []
                    for hp2 in range(2):
                        av = avp.tile([DH + 1, 512], f32, tag="av",
                                      name=f"av_{hc}_{qc}_{hp2}")
                        avs.append(av)
                    for g in range(qc + 1):
                        diag = (g == qc)
                        for half in range(2):
                            # (offset, width) of each k-block's valid
                            # q-span inside the p tile; diagonal blocks
                            # are clipped to q >= k_block_start
                            if diag:
                                rs = [2 * half, 2 * half + 1]
                                spans = [(128 * r, 512 - 128 * r)
                                         for r in rs]
                            else:
                                spans = [(0, 512), (0, 512)]
                            offs = [0, spans[0][1]]
                            scs = []
                            for hp2 in range(2):
                                sc = scp.tile([128, 1024], f32, tag="sc",
                                              name=f"sc_{hc}_{qc}_{g}_{half}_{hp2}")
                                scs.append(sc)
                            for r2 in range(2):
                                kb = 4 * g + 2 * half + r2
                                qo, w = spans[r2]
                                for hp2 in range(2):
                                    hp = hp2 * 64
                                    nc.tensor.matmul(
                                        scs[hp2][:, offs[r2]:offs[r2] + w],
                                        kT[hp:hp + 64, hc,
                                           kb * 128:(kb + 1) * 128],
                                        qT[hp:hp + 64, hc,
                                           qc * 512 + qo:(qc + 1) * 512],
                                        start=True, stop=True,
                                        tile_position=(hp, 0))
                            width = offs[1] + spans[1][1]
                            for hp2 in range(2):
                                h = hc * 2 + hp2
                                p_sb = ppool.tile([128, 1024], f16,
                                                  tag=f"p{hp2}",
                                                  name=f"p_{hc}_{qc}_{g}_{half}_{hp2}")
                                # the reference 1/sqrt(64) score scale
                                nc.scalar.activation(
                                    p_sb[:, 0:width],
                                    scs[hp2][:, 0:width], Exp,
                                    scale=0.125)
                                if diag:
                                    # only the first 128 columns of a
                                    # clipped block straddle the diagonal
                                    for r2 in range(2):
                                        nc.vector.tensor_mul(
                                            p_sb[:, offs[r2]:offs[r2] + 128],
                                            p_sb[:, offs[r2]:offs[r2] + 128],
                                            mask[:, 384:512])
                                for r2 in range(2):
                                    kb = 4 * g + 2 * half + r2
                                    qo, w = spans[r2]
                                    nc.tensor.matmul(
                                        avs[hp2][:, qo:512],
                                        v[:, kb, h, :],
                                        p_sb[:, offs[r2]:offs[r2] + w],
                                        start=(kb == 0),
                                        stop=(kb == 4 * qc + 3))
                            # hold the last two fillers back on the final
                            # diagonal group: they instead bridge the
                            # tensor-idle window between the last AV and
                            # the tail norm/wo chain (an idle dip there
                            # also triggers a ~10us PE half-clock clamp)
                            if not (last and hc == 1 and diag):
                                run_filler(1)
                    if last and hc == 1:
                        # tail epilogue: the l-row copies go FIRST (the
                        # K=1 transpose matmuls below wait on them), split
                        # across DVE and ACT so they run in parallel; the
                        # outT copies follow (their consumers come later)
                        nc.vector.tensor_copy(
                            l_row16[0:1, 0:512], avs[0][DH:DH + 1, :])
                        nc.scalar.copy(
                            l_row16[0:1, 512:1024], avs[1][DH:DH + 1, :])
                        # held-back fillers go on the tensor queue first
                        # so the PE isn't idle while the copies land
                        run_filler(len(fillers))
                        # lT_ps[p, hp2*4+t] = l_row16[0, hp2*512+128t+p]:
                        # K=1 matmul with the single l row as stationary
                        # and a 1.0 scalar as the moving operand; PSUM
                        # comes from the score pool (idle from here on)
                        # so the ybc ring stays free for the tail wo
                        ltp = scp.tile([128, 1024], f32, tag="sc",
                                       name="ltp")[:, 0:8]
                        for hp2 in range(2):
                            for t4 in range(4):
                                nc.tensor.matmul(
                                    ltp[:, hp2 * 4 + t4:hp2 * 4 + t4 + 1],
                                    l_row16[0:1, hp2 * 512 + t4 * 128:
                                            hp2 * 512 + (t4 + 1) * 128],
                                    warm[0:1, 0:1],
                                    start=True, stop=True)
                        att_hc.ltp = ltp
                        # reciprocals immediately (the bc broadcast
                        # matmuls in the tail norm wait on them)
                        with nc.allow_low_precision(
                                reason="fp16 1/l; l>=1 so ~5e-4 relative"):
                            for hp2 in range(2):
                                h = hc * 2 + hp2
                                cols = slice(h * NT + 4 * qc,
                                             h * NT + 4 * qc + 4)
                                nc.vector.reciprocal(
                                    recipT16[:, cols],
                                    ltp[:, hp2 * 4:hp2 * 4 + 4])
                        # outT copies in 256-column halves, split across
                        # ACT/DVE, so the first tail normalization chunk
                        # isn't gated on a whole 512-wide copy
                        for ci in range(2):
                            cs = slice(qc * 512 + ci * 256,
                                       qc * 512 + (ci + 1) * 256)
                            vs = slice(ci * 256, (ci + 1) * 256)
                            nc.scalar.copy(
                                outT[0:64, hc, cs], avs[0][0:DH, vs])
                            nc.vector.tensor_copy(
                                outT[64:128, hc, cs], avs[1][0:DH, vs])
                    else:
                        for hp2 in range(2):
                            h = hc * 2 + hp2
                            nc.vector.tensor_copy(
                                outT[hp2 * 64:hp2 * 64 + 64, hc,
                                     qc * 512:(qc + 1) * 512],
                                avs[hp2][0:DH, :])
                            # denominators: need the 512 l values spread
                            # across 128 partitions (a [1,512] single-lane
                            # DVE reciprocal measures 3.3us; the [128,4]
                            # one is ~0.15us); the roundtrip through DRAM
                            # is fully overlapped in steady state
                            seg = slice(h * S + qc * 512,
                                        h * S + (qc + 1) * 512)
                            nc.vector.tensor_copy(
                                l_row[0:1, seg], avs[hp2][DH:DH + 1, :])
                            nc.sync.dma_start(l_dram[seg], l_row[0:1, seg])
                            nc.sync.dma_start(
                                lT[:, h * NT + 4 * qc:h * NT + 4 * qc + 4],
                                l_dram[seg].rearrange("(t p) -> p t", p=128))

                def norm_pair(qc, hc):
                    # 1/l on the [q-partition] transposed copy (128 DVE
                    # lanes), broadcast over the dh rows with K=128 ident
                    # matmuls -- the two heads packed onto PE column halves
                    # via tile_position -- then one tensor_mul normalizes
                    # the whole [128,512] chunk
                    sl = slice(qc * 512, (qc + 1) * 512)
                    from_ltp = (qc == NQ - 1 and hc == 1)
                    if not from_ltp:
                        with nc.allow_low_precision(
                                reason="fp16 1/l; l>=1 so ~5e-4 relative"):
                            for hp2 in range(2):
                                h = hc * 2 + hp2
                                cols = slice(h * NT + 4 * qc,
                                             h * NT + 4 * qc + 4)
                                nc.vector.reciprocal(recipT16[:, cols],
                                                     lT[:, cols])
                    if from_ltp:
                        # the ybc ring is reserved for the tail wo PSUM;
                        # the score pool is idle from here on
                        bc = scp.tile([128, 1024], f32, tag="sc",
                                      name=f"bc_{hc}_{qc}")[:, 0:512]
                    else:
                        bc = ybcp.tile([128, 512], f32, tag="ybc",
                                       name=f"bc_{hc}_{qc}")
                    for hp2 in range(2):
                        for t4 in range(4):
                            col = (hc * 2 + hp2) * NT + 4 * qc + t4
                            nc.tensor.matmul(
                                bc[hp2 * 64:(hp2 + 1) * 64,
                                   t4 * 128:(t4 + 1) * 128],
                                recipT16[:, col:col + 1]
                                .to_broadcast((128, DH)),
                                ident[:],
                                start=True, stop=True,
                                tile_position=(0, hp2 * 64))
                    if from_ltp:
                        # final chunk: normalize in 128-column pieces so
                        # the first tail wo matmul isn't gated on the
                        # whole 512-wide multiply
                        for t4 in range(4):
                            s4 = slice(qc * 512 + t4 * 128,
                                       qc * 512 + (t4 + 1) * 128)
                            nc.vector.tensor_mul(
                                outT[:, hc, s4], outT[:, hc, s4],
                                bc[:, t4 * 128:(t4 + 1) * 128])
                    else:
                        nc.vector.tensor_mul(
                            outT[:, hc, sl], outT[:, hc, sl], bc[:])

                ysb_map = {}

                def wo_oc(qt, oc, tail=False):
                    if qt not in ysb_map:
                        ysb_map[qt] = ysb_pool.tile(
                            [128, 1024], f16, tag="ysb", name=f"ysb_{qt}")
                    ysb = ysb_map[qt]
                    yps = ybcp.tile([128, 512], f32, tag="ybc",
                                    name=f"yps_{qt}_{oc}")
                    for fc in range(FC):
                        nc.tensor.matmul(
                            yps[:],
                            outT[:, fc, qt * 128:(qt + 1) * 128],
                            wo[:, fc, oc * 512:(oc + 1) * 512],
                            start=(fc == 0), stop=(fc == FC - 1))
                    if tail and oc == 0:
                        nc.scalar.copy(
                            ysb[:, oc * 512:(oc + 1) * 512], yps[:])
                    else:
                        nc.vector.tensor_copy(
                            ysb[:, oc * 512:(oc + 1) * 512], yps[:])
                    if tail and qt == 4 * NQ - 1:
                        # last token block: ship each half as soon as its
                        # cast lands so the final DMA isn't serialized
                        # behind both halves
                        nc.sync.dma_start(
                            y_d[qt * 128:(qt + 1) * 128,
                                oc * 512:(oc + 1) * 512],
                            ysb[:, oc * 512:(oc + 1) * 512])
                    elif oc == 1:
                        nc.sync.dma_start(
                            y_d[qt * 128:(qt + 1) * 128, :], ysb[:])

                def wo_qt(qt, tail=False):
                    for oc in range(2):
                        wo_oc(qt, oc, tail=tail)

                proj_qk0()
                for tt in range(4):
                    proj_v_group(tt, big=(tt % 2 == 0))
                for qc in range(NQ):
                    if qc + 1 < NQ:
                        nxt = qc + 1
                        if nxt < NQ - 1:
                            # project everything for the next chunk now
                            for w_sb, dstT in ((wq, qT), (wk, kT)):
                                for fc in range(FC):
                                    fillers.append(
                                        lambda w=w_sb, d=dstT, f=fc, t=nxt:
                                        proj_qk_group(w, d, f, t))
                            for tt in range(4 * nxt, 4 * nxt + 4):
                                fillers.append(lambda t=tt: proj_v_group(t))
                        else:
                            # last chunk: only qT is needed at att(3) g=0;
                            # kT + v for kb 12-15 are deferred into att(3)
                            # itself (first touched at g=3) to keep the
                            # filler queue alive there
                            for fc in range(FC):
                                fillers.append(
                                    lambda f=fc, t=nxt:
                                    proj_qk_group(wq, qT, f, t))
                    if qc >= 1:
                        for hcx in range(FC):
                            fillers.append(
                                lambda q=qc - 1, c=hcx: norm_pair(q, c))
                        if qc == 1:
                            for qt in range(0, 4):
                                fillers.append(lambda a=qt: wo_qt(a))
                        else:
                            for qt in range(4 * (qc - 1), 4 * qc):
                                for oc in range(2):
                                    fillers.append(
                                        lambda a=qt, o=oc: wo_oc(a, o))
                    if qc == NQ - 1:
                        # deferred last-chunk projections, due before g=3
                        deferred = []
                        for fc in range(FC):
                            deferred.append(
                                lambda f=fc, t=qc:
                                proj_qk_group(wk, kT, f, t))
                        for tt in range(4 * qc, 4 * qc + 4):
                            deferred.append(lambda t=tt: proj_v_group(t))
                        fillers.extendleft(reversed(deferred))
                    att_hc(qc, 0)
                    if qc == NQ - 1:
                        # norm(3,0) inside att(3,1), late enough that its
                        # l-transpose DMAs (issued at the end of att(3,0))
                        # have landed by the time the reciprocal runs
                        fillers.insert(min(4, len(fillers)),
                                       lambda: norm_pair(NQ - 1, 0))
                    att_hc(qc, 1)
                    run_filler(len(fillers))
                # tail wo: interleave token-block pairs so a ready fc0
                # matmul always sits between the normalization-gated fc1
                # matmuls; casts alternate ACT/DVE; the last block ships
                # per-half.  The first pair's fc0 matmuls are hoisted
                # ahead of the tail norm so the PE stays busy while the
                # reciprocals land.
                for qt in range(4 * (NQ - 1), 4 * NQ):
                    ysb_map[qt] = ysb_pool.tile([128, 1024], f16,
                                                tag="ysb", name=f"ysb_{qt}")
                yp = {}

                def wo_fc0(qt, oc, pool="ybc"):
                    if pool == "sc":
                        yp[(qt, oc)] = scp.tile(
                            [128, 1024], f32, tag="sc",
                            name=f"ypt_{qt}_{oc}")[:, 0:512]
                    else:
                        yp[(qt, oc)] = ybcp.tile(
                            [128, 512], f32, tag="ybc",
                            name=f"ypt_{qt}_{oc}")
                    nc.tensor.matmul(
                        yp[(qt, oc)][:],
                        outT[:, 0, qt * 128:(qt + 1) * 128],
                        wo[:, 0, oc * 512:(oc + 1) * 512],
                        start=True, stop=False)

                b0 = 4 * (NQ - 1)
                wo_fc0(b0, 0)
                wo_fc0(b0 + 1, 0)
                norm_pair(NQ - 1, 1)
                # third and fourth fc0 matmuls ride the score pool (its
                # ltp/bc slots are read-complete by then), so the PE has
                # ready work while the per-chunk normalizations drain
                wo_fc0(b0 + 2, 0, pool="sc")
                wo_fc0(b0 + 3, 0, pool="sc")
                for base in (b0, b0 + 2):
                    for oc in range(2):
                        for qt in (base, base + 1):
                            if (qt, oc) not in yp:
                                wo_fc0(qt, oc)
                        for qt in (base, base + 1):
                            nc.tensor.matmul(
                                yp[(qt, oc)][:],
                                outT[:, 1, qt * 128:(qt + 1) * 128],
                                wo[:, 1, oc * 512:(oc + 1) * 512],
                                start=False, stop=True)
                        for qt in (base, base + 1):
                            dst = ysb_map[qt][:, oc * 512:(oc + 1) * 512]
                            if qt == 4 * NQ - 1 and oc == 1:
                                # final piece: 256-wide halves on both
                                # engines, shipped separately, so the
                                # last DMA starts as early as possible
                                nc.scalar.copy(
                                    dst[:, 0:256], yp[(qt, oc)][:, 0:256])
                                nc.vector.tensor_copy(
                                    dst[:, 256:512],
                                    yp[(qt, oc)][:, 256:512])
                                nc.sync.dma_start(
                                    y_d[qt * 128:(qt + 1) * 128,
                                        512:768], dst[:, 0:256])
                                nc.sync.dma_start(
                                    y_d[qt * 128:(qt + 1) * 128,
                                        768:1024], dst[:, 256:512])
                                continue
                            if (qt + oc) % 2 == 0:
                                nc.scalar.copy(dst, yp[(qt, oc)][:])
                            else:
                                nc.vector.tensor_copy(dst, yp[(qt, oc)][:])
                            if qt == 4 * NQ - 1:
                                nc.sync.dma_start(
                                    y_d[qt * 128:(qt + 1) * 128,
                                        oc * 512:(oc + 1) * 512], dst)
                            elif oc == 1:
                                nc.sync.dma_start(
                                    y_d[qt * 128:(qt + 1) * 128, :],
                                    ysb_map[qt][:])

    nc.compile()

    from concourse.bass_interp import get_hw_module
    nc.m = get_hw_module(nc.m)

    _CACHE[key] = nc
    return nc


def kernel(x, wq, wk, wv, wo):
    x = np.asarray(x, dtype=np.float32)
    wq = np.asarray(wq, dtype=np.float32)
    wk = np.asarray(wk, dtype=np.float32)
    wv = np.asarray(wv, dtype=np.float32)
    wo = np.asarray(wo, dtype=np.float32)

    from concourse import bass_utils

    nc = _build_program()

    def sbuf_w(w):
        # [out=256, in=1024] -> [128, DC, 256] SBUF layout, contiguous DMA
        return np.ascontiguousarray(
            w.T.reshape(DC, 128, F).transpose(1, 0, 2)).astype(np.float16)

    in_maps = []
    for c in range(8):
        b = c // 4
        hg = c % 4
        fs = slice(hg * F, (hg + 1) * F)
        # [NQ, 128, DC, 512]: stripe-major for early projection start
        xT = np.ascontiguousarray(
            x[b].T.reshape(DC, 128, NQ, 512).transpose(2, 1, 0, 3)
        ).astype(np.float16)
        woT = np.ascontiguousarray(
            wo[:, fs].T.reshape(FC, 128, D).transpose(1, 0, 2)
        ).astype(np.float16)
        in_maps.append({
            "xT": xT,
            "wq": sbuf_w(wq[fs, :]),
            "wk": sbuf_w(wk[fs, :]),
            "wv": sbuf_w(wv[fs, :]),
            "wo": woT,
        })

    res = bass_utils.run_bass_kernel_spmd(nc, in_maps, core_ids=list(range(8)))
    ys = [res.results[c]["y"].astype(np.float32) for c in range(8)]
    out = np.stack([ys[0] + ys[1] + ys[2] + ys[3],
                    ys[4] + ys[5] + ys[6] + ys[7]])
    return out
